# revision 4
# baseline (speedup 1.0000x reference)
"""Trainium2 Bass kernel for CrossEfficientAttention (B=8, C=256, H=W=64, 4 heads).

Sharding: data-parallel over batch B — one sample per NeuronCore, no collectives.

Per-core math (sample x_s, c_s of shape [C, N], N = H*W = 4096):
    Q  = wq @ x_s                      (+ bq, folded into the exp's ACT bias)
    KV = wkv @ c_s                     (bkv[:C] cancels exactly in softmax over N;
                                        bkv[C:] handled as a rank-1 update of W)
    k  = softmax_N(K); q = softmax_head(Q * C**-0.25)
    context = k @ V^T ; out = wo @ (context @ q) + bo

Restructured for the PE array (out = lhsT.T @ rhs, contraction over partitions):
  * KV^T computed directly in [N, C] layout by using c_s tiles as lhsT.
  * k-softmax normalizer: ones-columns appended to V^T give row sums of exp(K)
    in column 256 of the context PSUM accumulator; context rows are then scaled
    by the reciprocal column (per-partition tensor_scalar) — no transposes.
  * wo folded in early: W^T = matmul(lhsT=context, rhs=wo^T) directly in [d, o]
    layout. The per-chunk output is then just out2 = W^T.T @ q.
  * q-softmax denominators: a block-diagonal indicator matmul (B[c,c'] = 1 iff
    same head) sums exp(Q) directly into a partition-replicated [128, 512] PSUM
    tile per half, so 1/D = exp(-ln D) on ScalarE feeds the qt multiply with no
    separate broadcast matmul.

All matmul operands are bfloat16 (PE runs bf16 at the same 1 cycle/row as
float32r, but HBM traffic and SBUF footprint halve and DVE/ACT run 2x);
PSUM accumulation stays fp32, and the output rides back to the host as bf16
(chunk-contiguous layout so store descriptors are 2 KB/partition-row) and is
upcast + reordered there. Input streaming is ordered so the KV phase's cf
tiles always lead the x tiles on the sync DGE queue.
"""

import numpy as np
import ml_dtypes

import concourse.bass as bass
import concourse.tile as tile
from concourse import bacc, mybir
from concourse.bass import ts
from concourse.bass_utils import run_bass_kernel_spmd

B, C, H, W = 8, 256, 64, 64
N = H * W
NHEADS = 4
DHEAD = C // NHEADS
NCORES = 8
NSUPER = N // 256          # 16 double-n-tile iterations for the KV phase
NCHUNKS = N // 512         # 8 column chunks for the Q/output phase
SCALE = float(1.0 / np.sqrt(np.sqrt(np.float32(C))))
VW = C + 2                 # V^T tile row width (256 data + 2 ones cols)
# packed weight row width per c-half: wkvT|wqT|woT|ind
WP = 2 * C + C + C + NHEADS
NWARM = 6                  # PE-ramp warmup matmuls (fill the initial DMA wait)

F32 = mybir.dt.float32
F32R = mybir.dt.float32r
BF16 = mybir.dt.bfloat16
AF = mybir.ActivationFunctionType

_CACHE = {}


def _single_act_table():
    """Scope-patch the activation-table list so the table-load pass resolves
    both Exp and Ln to natural_log_exp_and_others (set ids stay positional,
    so only the function lists may change, not the order)."""
    import contextlib

    import concourse.bacc as cbacc
    from concourse.hw_specs import get_activation_tables

    @contextlib.contextmanager
    def scope():
        orig = cbacc.get_activation_tables

        def patched(arch):
            tabs = get_activation_tables(arch)
            return {
                k: (v if k == "natural_log_exp_and_others" else set())
                for k, v in tabs.items()
            }

        cbacc.get_activation_tables = patched
        try:
            yield
        finally:
            cbacc.get_activation_tables = orig

    return scope()


def _build(use_bq, use_bo, use_bv, mm_dtype):
    nc = bacc.Bacc("TRN2", target_bir_lowering=False, debug=False)
    MDT = mm_dtype
    YDT = BF16 if MDT == BF16 else F32

    x = nc.dram_tensor("x", [C, N], MDT, kind="ExternalInput")
    cp = nc.dram_tensor("cp", [C, N], MDT, kind="ExternalInput")
    wpack = nc.dram_tensor("wpack", [128, 2 * WP], MDT, kind="ExternalInput")
    sel = nc.dram_tensor("sel", [NHEADS, C], MDT, kind="ExternalInput")
    if use_bq:
        bq_s = nc.dram_tensor("bq_s", [C, 1], F32, kind="ExternalInput")
    if use_bo:
        bo_c = nc.dram_tensor("bo_c", [C, 1], F32, kind="ExternalInput")
    if use_bv:
        bv_r = nc.dram_tensor("bv_r", [1, C], MDT, kind="ExternalInput")
        wosum = nc.dram_tensor("wosum", [1, C], MDT, kind="ExternalInput")
    # chunk-contiguous output: y3[p, j, t, s] = out[t*128+p, j*512+s]
    y = nc.dram_tensor("y", [128, NCHUNKS * 2 * 512], YDT, kind="ExternalOutput")
    y4 = y[:].rearrange("p (j t s) -> p j t s", j=NCHUNKS, t=2)

    with tile.TileContext(nc) as tc:
        with (
            tc.tile_pool(name="const", bufs=1) as cst,
            tc.tile_pool(name="big", bufs=1) as big,
            tc.tile_pool(name="qsb", bufs=4) as qsb,
            tc.tile_pool(name="dsb", bufs=3) as dsb,
        ):
            # HAM warmup scratch: memset on gpsimd (idle queue) so the PE can
            # start ramping as soon as the framework preamble ends.
            scratch = cst.tile([128, 512], MDT, name="scratch")
            nc.gpsimd.memset(scratch[:], 1.0)

            # --- packed weights; the KV-phase slice (wkvT) rides first ---
            wpk = cst.tile([128, 2 * WP], MDT, name="wpk")
            wpk3 = wpk[:].rearrange("p (u w) -> p u w", u=2)
            wpack3 = wpack[:].rearrange("p (u w) -> p u w", u=2)
            wkvT_sb = [wpk[:, u * WP : u * WP + 2 * C] for u in range(2)]
            wqT_sb = [wpk[:, u * WP + 2 * C : u * WP + 3 * C] for u in range(2)]
            woT_sb = [wpk[:, u * WP + 3 * C : u * WP + 4 * C] for u in range(2)]
            ind_sb = [wpk[:, u * WP + 4 * C : u * WP + 4 * C + NHEADS] for u in range(2)]
            sel_sb = [cst.tile([NHEADS, 128], MDT, name=f"sel{u}") for u in range(2)]

            # weight-side loads ride the scalar DGE queue
            nc.scalar.dma_start(out=wpk3[:, :, 0 : 2 * C], in_=wpack3[:, :, 0 : 2 * C])
            for u in range(2):
                nc.scalar.dma_start(out=sel_sb[u][:], in_=sel[:, ts(u, 128)])
            if use_bq:
                bq_sb = [cst.tile([128, 1], F32, name=f"bq{u}") for u in range(2)]
                for u in range(2):
                    nc.scalar.dma_start(out=bq_sb[u][:], in_=bq_s[ts(u, 128), :])
            if use_bo:
                bo_sb = [cst.tile([128, 1], F32, name=f"bo{u}") for u in range(2)]
                for u in range(2):
                    nc.scalar.dma_start(out=bo_sb[u][:], in_=bo_c[ts(u, 128), :])
            if use_bv:
                bv_sb = cst.tile([1, C], MDT, name="bv_sb")
                nc.scalar.dma_start(out=bv_sb[:], in_=bv_r[:])
                wosum_sb = cst.tile([1, C], MDT, name="wosum_sb")
                nc.scalar.dma_start(out=wosum_sb[:], in_=wosum[:])
            nc.scalar.dma_start(out=wpk3[:, :, 2 * C : WP], in_=wpack3[:, :, 2 * C : WP])

            # cf stream leads on the sync queue (the KV phase consumes it in
            # order); x follows on the same queue so it can never compete with
            # the latency-critical early cf chunks. Chunks are >=1024 cols
            # after the first so descriptor rows stay >=2KB in bf16.
            cf_sb = [big.tile([128, N], MDT, name=f"cf{u}") for u in range(2)]
            xf_sb = [big.tile([128, N], MDT, name=f"xf{u}") for u in range(2)]
            for c0, c1 in ((0, 256), (256, 1024), (1024, 2048), (2048, 4096)):
                for u in range(2):
                    nc.sync.dma_start(
                        out=cf_sb[u][:, c0:c1], in_=cp[ts(u, 128), c0:c1]
                    )
            for c0, c1 in ((0, 2048), (2048, 4096)):
                for u in range(2):
                    nc.sync.dma_start(
                        out=xf_sb[u][:, c0:c1], in_=x[ts(u, 128), c0:c1]
                    )

            # persistent W^T tiles (filled in the epilogue)
            WT_sb = [cst.tile([128, C], MDT, name=f"WT{u}") for u in range(2)]

            # HAM warmup: dependency-free matmuls on scratch data keep the
            # PE busy/ramping during the initial DMA wait.
            with tc.tile_pool(name="pswarm", bufs=1, space="PSUM") as pwm:
                pswarm = pwm.tile([128, 512], F32, name="pswarm")
                for _ in range(NWARM):
                    nc.tensor.matmul(
                        pswarm[:], scratch[:, 0:128], scratch[:],
                        start=True, stop=True, skip_group_check=True,
                    )

            # manually-rotated V^T ring: ones columns pre-set once
            NVBUF = 4
            v2r = [cst.tile([128, 2 * VW], MDT, name=f"v2_{i}") for i in range(NVBUF)]
            for i in range(NVBUF):
                for h in range(2):
                    nc.vector.memset(v2r[i][:, h * VW + C : h * VW + C + 2], 1.0)

            eqs, psDs, rDs, psRbs, qts, psOs = {}, {}, {}, {}, {}, {}

            def q_mms_into(j, psQ):
                for t in range(2):
                    for u in range(2):
                        nc.tensor.matmul(
                            psQ[:, t * 512 : (t + 1) * 512],
                            wqT_sb[u][:, ts(t, 128)],
                            xf_sb[u][:, ts(j, 512)],
                            start=(u == 0),
                            stop=(u == 1),
                        )

            def eq_act(j, psQ):
                eq = qsb.tile([128, 1024], MDT, name="eq", tag="eq")
                if use_bq:
                    for t in range(2):
                        nc.scalar.activation(
                            out=eq[:, t * 512 : (t + 1) * 512],
                            in_=psQ[:, t * 512 : (t + 1) * 512],
                            func=AF.Exp,
                            scale=SCALE,
                            bias=bq_sb[t][:],
                        )
                else:
                    nc.scalar.activation(
                        out=eq[:], in_=psQ[:], func=AF.Exp, scale=SCALE
                    )
                eqs[j] = eq

            # ============ KV phase: context = exp(K) @ [V^T | 1] ============
            # Software-pipelined by one iteration: the PE runs iteration i's
            # KV matmuls and iteration i-1's context matmuls back to back.
            with tc.tile_pool(name="psum_ctx", bufs=1, space="PSUM") as pctx:
                psCtx = [
                    pctx.tile([128, C + 2], F32, name=f"psCtx{u}") for u in range(2)
                ]
                with (
                    tc.tile_pool(name="psum_kv", bufs=3, space="PSUM") as pkv,
                    tc.tile_pool(name="kvsb", bufs=3) as kvsb,
                ):
                    eks = {}

                    def kv_mms(i):
                        psKV = pkv.tile([128, 1024], F32, name="psKV")
                        for h in range(2):
                            nt = 2 * i + h
                            for u in range(2):
                                nc.tensor.matmul(
                                    psKV[:, h * 512 : (h + 1) * 512],
                                    cf_sb[u][:, ts(nt, 128)],
                                    wkvT_sb[u],
                                    start=(u == 0),
                                    stop=(u == 1),
                                )
                        return psKV

                    def ctx_mms(i):
                        ek = eks.pop(i)
                        v2 = v2r[i % NVBUF]
                        for h in range(2):
                            for u in range(2):
                                nc.tensor.matmul(
                                    psCtx[u][:],
                                    ek[:, h, ts(u, 128)],
                                    v2[:, h * VW : (h + 1) * VW],
                                    start=(i == 0 and h == 0),
                                    stop=(i == NSUPER - 1 and h == 1),
                                    skip_group_check=True,
                                )

                    def kv_post(i, psKV):
                        ek = kvsb.tile([128, 2, C], MDT, name="ek")
                        nc.scalar.activation(
                            out=ek[:],
                            in_=psKV[:].rearrange("p (h c) -> p h c", h=2)[:, :, 0:C],
                            func=AF.Exp,
                        )
                        eks[i] = ek
                        v2 = v2r[i % NVBUF]
                        nc.vector.tensor_copy(
                            v2[:].rearrange("p (h w) -> p h w", h=2)[:, :, 0:C],
                            psKV[:].rearrange("p (h c) -> p h c", h=2)[:, :, C : 2 * C],
                        )

                    for i in range(NSUPER):
                        psKV = kv_mms(i)
                        if i > 1:
                            ctx_mms(i - 2)
                        kv_post(i, psKV)
                    ctx_mms(NSUPER - 2)
                    ctx_mms(NSUPER - 1)
                    # overlap the context epilogue with the first two Q chunks
                    # (their PSUM supertiles borrow the KV pool's slots)
                    for j in range(2):
                        psQ = pkv.tile([128, 1024], F32, name="psKV", tag="psKV")
                        q_mms_into(j, psQ)
                        eq_act(j, psQ)

                # ===== epilogue: normalize context, fold wo: W^T = ctx.T@woT =====
                rcol = [cst.tile([128, 1], F32, name=f"rcol{u}") for u in range(2)]
                ctx_sb = [cst.tile([128, C], MDT, name=f"ctx{u}") for u in range(2)]
                for u in range(2):
                    nc.vector.reciprocal(rcol[u][:], psCtx[u][:, C : C + 1])
                    nc.vector.tensor_scalar_mul(
                        out=ctx_sb[u][:], in0=psCtx[u][:, 0:C], scalar1=rcol[u][:]
                    )
                with tc.tile_pool(name="psum_w", bufs=1, space="PSUM") as pw:
                    psW = [pw.tile([128, C], F32, name=f"psW{v}") for v in range(2)]
                    for v in range(2):
                        for u in range(2):
                            nc.tensor.matmul(
                                psW[v][:],
                                ctx_sb[u][:, ts(v, 128)],
                                woT_sb[u],
                                start=(u == 0),
                                stop=(u == 1) and not use_bv,
                                skip_group_check=True,
                            )
                        if use_bv:
                            # context gains +bv[d'] per row (sum_n k = 1), so
                            # W^T += bv (X) rowsum(wo): a K=1 rank-1 matmul.
                            nc.tensor.matmul(
                                psW[v][:],
                                bv_sb[:, ts(v, 128)],
                                wosum_sb[:],
                                start=False,
                                stop=True,
                                skip_group_check=True,
                            )
                        nc.vector.tensor_copy(WT_sb[v][:], psW[v][:])

            # ============ Q phase: out = W^T.T @ softmax_head(exp(Q*s)) ============
            # Supertile layout [128, 1024]: channel-half t at cols 512t.
            # Pipelined depth 3: at iteration j the PE runs Q(j), D(j-1),
            # out(j-3) so every matmul's ACT/DVE inputs are a full iteration
            # old. D comes out of the head-block indicator matmul already
            # replicated over the 128 partitions of each half.
            with (
                tc.tile_pool(name="psq", bufs=1, space="PSUM") as pq,
                tc.tile_pool(name="psd", bufs=1, space="PSUM") as pd,
                tc.tile_pool(name="psrb", bufs=1, space="PSUM") as prb,
                tc.tile_pool(name="pso", bufs=1, space="PSUM") as po,
            ):
                def q_mms(j):
                    psQ = pq.tile([128, 1024], F32, name="psQ")
                    q_mms_into(j, psQ)
                    return psQ

                def d_mms(j):
                    psD = pd.tile([NHEADS, 512], F32, name="psD")
                    for t in range(2):
                        nc.tensor.matmul(
                            psD[:],
                            ind_sb[t],
                            eqs[j][:, t * 512 : (t + 1) * 512],
                            start=(t == 0),
                            stop=(t == 1),
                        )
                    psDs[j] = psD

                def r_acts(j):
                    lnD = dsb.tile([NHEADS, 512], F32, name="lnD")
                    nc.scalar.activation(out=lnD[:], in_=psDs.pop(j)[:], func=AF.Ln)
                    rD = dsb.tile([NHEADS, 512], MDT, name="rD")
                    nc.scalar.activation(out=rD[:], in_=lnD[:], func=AF.Exp, scale=-1.0)
                    rDs[j] = rD

                def rb_mms(j):
                    psRb = prb.tile([128, 1024], F32, name="psRb")
                    rD = rDs.pop(j)
                    for t in range(2):
                        nc.tensor.matmul(
                            psRb[:, t * 512 : (t + 1) * 512],
                            sel_sb[t][:],
                            rD[:],
                            start=True,
                            stop=True,
                        )
                    psRbs[j] = psRb

                def q_mul(j):
                    qt = qsb.tile([128, 1024], MDT, name="qt", tag="qt")
                    nc.vector.tensor_mul(qt[:], eqs.pop(j)[:], psRbs.pop(j)[:])
                    qts[j] = qt

                def out_mms(j):
                    psO = po.tile([128, 1024], F32, name="psO")
                    qt = qts.pop(j)
                    for t in range(2):
                        for u in range(2):
                            nc.tensor.matmul(
                                psO[:, t * 512 : (t + 1) * 512],
                                WT_sb[u][:, ts(t, 128)],
                                qt[:, u * 512 : (u + 1) * 512],
                                start=(u == 0),
                                stop=(u == 1),
                            )
                    psOs[j] = psO

                def store(j):
                    psO = psOs.pop(j)
                    o2 = qsb.tile([128, 1024], YDT, name="o2", tag="o2")
                    if use_bo:
                        for t in range(2):
                            nc.vector.tensor_scalar_add(
                                out=o2[:, t * 512 : (t + 1) * 512],
                                in0=psO[:, t * 512 : (t + 1) * 512],
                                scalar1=bo_sb[t][:],
                            )
                        nc.sync.dma_start(out=y4[:, j], in_=o2[:])
                    elif j == NCHUNKS - 1:
                        # final chunk: copy+store halves so the last DMA (and
                        # the tail barrier behind it) starts sooner
                        for t in range(2):
                            nc.vector.tensor_copy(
                                o2[:, t * 512 : (t + 1) * 512],
                                psO[:, t * 512 : (t + 1) * 512],
                            )
                            nc.sync.dma_start(
                                out=y4[:, j, t], in_=o2[:, t * 512 : (t + 1) * 512]
                            )
                    else:
                        nc.vector.tensor_copy(o2[:], psO[:])
                        nc.sync.dma_start(out=y4[:, j], in_=o2[:])

                for j in range(NCHUNKS + 3):
                    if 2 <= j < NCHUNKS:
                        psQ = q_mms(j)
                    if 1 <= j <= NCHUNKS:
                        d_mms(j - 1)
                    if 2 <= j <= NCHUNKS + 1:
                        rb_mms(j - 2)
                    if 3 <= j <= NCHUNKS + 2:
                        out_mms(j - 3)
                    if 2 <= j < NCHUNKS:
                        eq_act(j, psQ)
                    if 1 <= j <= NCHUNKS:
                        r_acts(j - 1)
                    if 2 <= j <= NCHUNKS + 1:
                        q_mul(j - 2)
                    if 3 <= j <= NCHUNKS + 2:
                        store(j - 3)

    nc.compile()
    return nc


def _get_nc(use_bq, use_bo, use_bv, mm_dtype):
    key = (use_bq, use_bo, use_bv, str(mm_dtype))
    if key not in _CACHE:
        with _single_act_table():
            _CACHE[key] = _build(use_bq, use_bo, use_bv, mm_dtype)
    return _CACHE[key]


def kernel(x, cproj, wq, bq, wkv, bkv, wo, bo, _mm_dtype=BF16, _results_hook=None):
    x = np.ascontiguousarray(np.asarray(x, dtype=np.float32).reshape(B, C, N))
    cf = np.ascontiguousarray(np.asarray(cproj, dtype=np.float32).reshape(B, C, N))
    wq = np.asarray(wq, dtype=np.float32)
    wkv = np.asarray(wkv, dtype=np.float32)
    wo = np.asarray(wo, dtype=np.float32)
    bq = np.asarray(bq, dtype=np.float32)
    bkv = np.asarray(bkv, dtype=np.float32)
    bo = np.asarray(bo, dtype=np.float32)

    use_bq = bool(np.any(bq != 0))
    use_bo = bool(np.any(bo != 0))
    bv = bkv[C:]
    use_bv = bool(np.any(bv != 0))

    wqT = np.ascontiguousarray(wq.T)
    wkvT = np.ascontiguousarray(wkv.T)
    woT = np.ascontiguousarray(wo.T)
    ind = np.zeros((C, NHEADS), np.float32)
    ind[np.arange(C), np.arange(C) // DHEAD] = 1.0
    sel = np.ascontiguousarray(ind.T)

    # packed weights per c-half u: [wkvT | wqT | woT | ind]
    wpack = np.zeros((128, 2 * WP), np.float32)
    for u in range(2):
        r = slice(u * 128, (u + 1) * 128)
        wpack[:, u * WP : u * WP + 2 * C] = wkvT[r]
        wpack[:, u * WP + 2 * C : u * WP + 3 * C] = wqT[r]
        wpack[:, u * WP + 3 * C : u * WP + 4 * C] = woT[r]
        wpack[:, u * WP + 4 * C : u * WP + 4 * C + NHEADS] = ind[r]

    nc = _get_nc(use_bq, use_bo, use_bv, _mm_dtype)

    if _mm_dtype == BF16:
        mdt_np = ml_dtypes.bfloat16
        x = x.astype(mdt_np)
        cf = cf.astype(mdt_np)
        wpack = wpack.astype(mdt_np)
        sel = sel.astype(mdt_np)
    base = {"wpack": wpack, "sel": sel}
    if use_bq:
        base["bq_s"] = (SCALE * bq).reshape(C, 1)
    if use_bo:
        base["bo_c"] = bo.reshape(C, 1)
    if use_bv:
        bv_r = bv.reshape(1, C)
        wosum = wo.sum(axis=1).reshape(1, C)
        if _mm_dtype == BF16:
            bv_r = bv_r.astype(ml_dtypes.bfloat16)
            wosum = wosum.astype(ml_dtypes.bfloat16)
        base["bv_r"] = bv_r
        base["wosum"] = wosum

    in_maps = [dict(base, x=x[b], cp=cf[b]) for b in range(B)]
    res = run_bass_kernel_spmd(nc, in_maps, list(range(NCORES)))
    if _results_hook is not None:
        _results_hook(res)
    # y3[p, j, t, s] = out[t*128+p, j*512+s]
    out = np.empty((B, C, N), np.float32)
    for b in range(B):
        y3 = np.asarray(res.results[b]["y"], dtype=np.float32).reshape(
            128, NCHUNKS, 2, 512
        )
        out[b] = y3.transpose(2, 0, 1, 3).reshape(C, N)
    return out.reshape(B, C, H, W)


# revision 5
# speedup vs baseline: 1.0320x; 1.0320x over previous
"""Trainium2 Bass kernel for CrossEfficientAttention (B=8, C=256, H=W=64, 4 heads).

Sharding: data-parallel over batch B — one sample per NeuronCore, no collectives.

Per-core math (sample x_s, c_s of shape [C, N], N = H*W = 4096):
    Q  = wq @ x_s                      (+ bq, folded into the exp's ACT bias)
    KV = wkv @ c_s                     (bkv[:C] cancels exactly in softmax over N;
                                        bkv[C:] handled as a rank-1 update of W)
    k  = softmax_N(K); q = softmax_head(Q * C**-0.25)
    context = k @ V^T ; out = wo @ (context @ q) + bo

Restructured for the PE array (out = lhsT.T @ rhs, contraction over partitions):
  * KV^T computed directly in [N, C] layout by using c_s tiles as lhsT; the
    context accumulation runs two iterations behind the KV matmuls and its
    matmuls are interleaved between them so neither LDWEIGHTS nor the exp's
    ACT latency ever stalls the PE.
  * k-softmax normalizer: ones-columns appended to V^T give row sums of exp(K)
    in column 256 of the context PSUM accumulator; context rows are then scaled
    by the reciprocal column (per-partition tensor_scalar) — no transposes.
  * wo folded in early: W^T = matmul(lhsT=context, rhs=wo^T) directly in [d, o]
    layout. The per-chunk output is then just out2 = W^T.T @ q.
  * q-softmax denominators: a head-block indicator matmul (hb[c,c'] = 1 iff
    same head) sums exp(Q) directly into a partition-replicated [128, 512]
    PSUM tile per half, so 1/D = exp(-ln D) on ScalarE feeds the qt multiply
    with no separate broadcast matmul; lnD/rD ride in fp16 so the second ACT
    pass runs at the 16-bit rate.

All matmul operands are float16 (PE runs fp16 at the same 1 cycle/row as
float32r, HBM traffic and SBUF footprint halve, DVE/ACT run 2x, and the
11-bit mantissa keeps rounding noise ~4x below bf16); PSUM stays fp32, and
the output rides back to the host as fp16 (chunk-contiguous layout so store
descriptors are 2 KB/partition-row) and is upcast + reordered there. The
SCALE factor of the q softmax is pre-folded into wq on the host.
"""

import numpy as np

import concourse.bass as bass
import concourse.tile as tile
from concourse import bacc, mybir
from concourse.bass import ts
from concourse.bass_utils import run_bass_kernel_spmd

B, C, H, W = 8, 256, 64, 64
N = H * W
NHEADS = 4
DHEAD = C // NHEADS
NCORES = 8
NSUPER = N // 256          # 16 double-n-tile iterations for the KV phase
NCHUNKS = N // 512         # 8 column chunks for the Q/output phase
SCALE = float(1.0 / np.sqrt(np.sqrt(np.float32(C))))
VW = C + 2                 # V^T tile row width (256 data + 2 ones cols)
# packed weight row width per c-half: wkvT|wqT|woT|headblock
WP = 2 * C + C + C + 128
NWARM = 6                  # PE-ramp warmup matmuls (fill the initial DMA wait)

F32 = mybir.dt.float32
F32R = mybir.dt.float32r
BF16 = mybir.dt.bfloat16
FP16 = mybir.dt.float16
AF = mybir.ActivationFunctionType

_CACHE = {}


def _single_act_table():
    """Scope-patch the activation-table list so the table-load pass resolves
    both Exp and Ln to natural_log_exp_and_others (set ids stay positional,
    so only the function lists may change, not the order)."""
    import contextlib

    import concourse.bacc as cbacc
    from concourse.hw_specs import get_activation_tables

    @contextlib.contextmanager
    def scope():
        orig = cbacc.get_activation_tables

        def patched(arch):
            tabs = get_activation_tables(arch)
            return {
                k: (v if k == "natural_log_exp_and_others" else set())
                for k, v in tabs.items()
            }

        cbacc.get_activation_tables = patched
        try:
            yield
        finally:
            cbacc.get_activation_tables = orig

    return scope()


def _build(use_bq, use_bo, use_bv, mm_dtype):
    nc = bacc.Bacc("TRN2", target_bir_lowering=False, debug=False)
    MDT = mm_dtype
    HALF = MDT in (BF16, FP16)
    YDT = MDT if HALF else F32
    LDT = FP16 if HALF else F32   # lnD/rD staging dtype

    x = nc.dram_tensor("x", [C, N], MDT, kind="ExternalInput")
    cp = nc.dram_tensor("cp", [C, N], MDT, kind="ExternalInput")
    wpack = nc.dram_tensor("wpack", [128, 2 * WP], MDT, kind="ExternalInput")
    if use_bq:
        bq_s = nc.dram_tensor("bq_s", [C, 1], F32, kind="ExternalInput")
    if use_bo:
        bo_c = nc.dram_tensor("bo_c", [C, 1], F32, kind="ExternalInput")
    if use_bv:
        bv_r = nc.dram_tensor("bv_r", [1, C], MDT, kind="ExternalInput")
        wosum = nc.dram_tensor("wosum", [1, C], MDT, kind="ExternalInput")
    # chunk-contiguous output: y[p, j, t, s] = out[t*128+p, j*512+s]
    y = nc.dram_tensor("y", [128, NCHUNKS * 2 * 512], YDT, kind="ExternalOutput")
    y4 = y[:].rearrange("p (j t s) -> p j t s", j=NCHUNKS, t=2)

    with tile.TileContext(nc) as tc:
        with (
            tc.tile_pool(name="const", bufs=1) as cst,
            tc.tile_pool(name="big", bufs=1) as big,
            tc.tile_pool(name="qsb", bufs=4) as qsb,
            tc.tile_pool(name="dsb", bufs=3) as dsb,
        ):
            # HAM warmup scratch: memset on gpsimd (idle queue) so the PE can
            # start ramping as soon as the framework preamble ends.
            scratch = cst.tile([128, 512], MDT, name="scratch")
            nc.gpsimd.memset(scratch[:], 1.0)

            # --- packed weights; the KV-phase slice (wkvT) rides first ---
            wpk = cst.tile([128, 2 * WP], MDT, name="wpk")
            wpk3 = wpk[:].rearrange("p (u w) -> p u w", u=2)
            wpack3 = wpack[:].rearrange("p (u w) -> p u w", u=2)
            wkvT_sb = [wpk[:, u * WP : u * WP + 2 * C] for u in range(2)]
            wqT_sb = [wpk[:, u * WP + 2 * C : u * WP + 3 * C] for u in range(2)]
            woT_sb = [wpk[:, u * WP + 3 * C : u * WP + 4 * C] for u in range(2)]
            hb_sb = [wpk[:, u * WP + 4 * C : u * WP + 4 * C + 128] for u in range(2)]

            # weight-side loads ride the scalar DGE queue
            nc.scalar.dma_start(out=wpk3[:, :, 0 : 2 * C], in_=wpack3[:, :, 0 : 2 * C])
            if use_bq:
                bq_sb = [cst.tile([128, 1], F32, name=f"bq{u}") for u in range(2)]
                for u in range(2):
                    nc.scalar.dma_start(out=bq_sb[u][:], in_=bq_s[ts(u, 128), :])
            if use_bo:
                bo_sb = [cst.tile([128, 1], F32, name=f"bo{u}") for u in range(2)]
                for u in range(2):
                    nc.scalar.dma_start(out=bo_sb[u][:], in_=bo_c[ts(u, 128), :])
            if use_bv:
                bv_sb = cst.tile([1, C], MDT, name="bv_sb")
                nc.scalar.dma_start(out=bv_sb[:], in_=bv_r[:])
                wosum_sb = cst.tile([1, C], MDT, name="wosum_sb")
                nc.scalar.dma_start(out=wosum_sb[:], in_=wosum[:])
            nc.scalar.dma_start(out=wpk3[:, :, 2 * C : WP], in_=wpack3[:, :, 2 * C : WP])

            # cf stream leads on the sync queue (the KV phase consumes it in
            # order); x follows on the same queue so it can never compete with
            # the latency-critical early cf chunks. Chunks are >=1024 cols
            # after the first so descriptor rows stay >=2KB at 16-bit dtypes.
            cf_sb = [big.tile([128, N], MDT, name=f"cf{u}") for u in range(2)]
            xf_sb = [big.tile([128, N], MDT, name=f"xf{u}") for u in range(2)]
            for c0, c1 in ((0, 512), (512, 1024), (1024, 2048), (2048, 4096)):
                for u in range(2):
                    nc.sync.dma_start(
                        out=cf_sb[u][:, c0:c1], in_=cp[ts(u, 128), c0:c1]
                    )
            for c0, c1 in ((0, 2048), (2048, 4096)):
                for u in range(2):
                    nc.sync.dma_start(
                        out=xf_sb[u][:, c0:c1], in_=x[ts(u, 128), c0:c1]
                    )

            # persistent W^T tiles (filled in the epilogue)
            WT_sb = [cst.tile([128, C], MDT, name=f"WT{u}") for u in range(2)]

            # HAM warmup: dependency-free matmuls on scratch data keep the
            # PE busy/ramping during the initial DMA wait.
            with tc.tile_pool(name="pswarm", bufs=1, space="PSUM") as pwm:
                pswarm = pwm.tile([128, 512], F32, name="pswarm")
                for _ in range(NWARM):
                    nc.tensor.matmul(
                        pswarm[:], scratch[:, 0:128], scratch[:],
                        start=True, stop=True, skip_group_check=True,
                    )

            # manually-rotated V^T ring: ones columns pre-set once
            NVBUF = 4
            v2r = [cst.tile([128, 2 * VW], MDT, name=f"v2_{i}") for i in range(NVBUF)]
            for i in range(NVBUF):
                for h in range(2):
                    nc.vector.memset(v2r[i][:, h * VW + C : h * VW + C + 2], 1.0)

            eqs, psDs, rDs, qts, psOs = {}, {}, {}, {}, {}

            def q_mms_into(j, psQ):
                for t in range(2):
                    for u in range(2):
                        nc.tensor.matmul(
                            psQ[:, t * 512 : (t + 1) * 512],
                            wqT_sb[u][:, ts(t, 128)],
                            xf_sb[u][:, ts(j, 512)],
                            start=(u == 0),
                            stop=(u == 1),
                        )

            def eq_act(j, psQ):
                eq = qsb.tile([128, 1024], MDT, name="eq", tag="eq")
                if use_bq:
                    for t in range(2):
                        nc.scalar.activation(
                            out=eq[:, t * 512 : (t + 1) * 512],
                            in_=psQ[:, t * 512 : (t + 1) * 512],
                            func=AF.Exp,
                            bias=bq_sb[t][:],
                        )
                else:
                    nc.scalar.activation(out=eq[:], in_=psQ[:], func=AF.Exp)
                eqs[j] = eq

            # ============ KV phase: context = exp(K) @ [V^T | 1] ============
            # The context matmuls run two iterations behind the KV matmuls and
            # are interleaved between them on the PE queue.
            with tc.tile_pool(name="psum_ctx", bufs=1, space="PSUM") as pctx:
                psCtx = [
                    pctx.tile([128, C + 2], F32, name=f"psCtx{u}") for u in range(2)
                ]
                with (
                    tc.tile_pool(name="psum_kv", bufs=3, space="PSUM") as pkv,
                    tc.tile_pool(name="kvsb", bufs=3) as kvsb,
                ):
                    eks = {}

                    def ctx_mm_list(i):
                        if i < 0:
                            return []
                        ek = eks.pop(i)
                        v2 = v2r[i % NVBUF]

                        def mk(h, u):
                            def go():
                                nc.tensor.matmul(
                                    psCtx[u][:],
                                    ek[:, h, ts(u, 128)],
                                    v2[:, h * VW : (h + 1) * VW],
                                    start=(i == 0 and h == 0),
                                    stop=(i == NSUPER - 1 and h == 1),
                                    skip_group_check=True,
                                )

                            return go

                        return [mk(h, u) for h in range(2) for u in range(2)]

                    def kv_iter(i, ctx_i):
                        psKV = pkv.tile([128, 1024], F32, name="psKV")
                        cms = ctx_mm_list(ctx_i)
                        ci = 0
                        for h in range(2):
                            nt = 2 * i + h
                            for u in range(2):
                                nc.tensor.matmul(
                                    psKV[:, h * 512 : (h + 1) * 512],
                                    cf_sb[u][:, ts(nt, 128)],
                                    wkvT_sb[u],
                                    start=(u == 0),
                                    stop=(u == 1),
                                    skip_group_check=True,
                                )
                                if ci < len(cms):
                                    cms[ci]()
                                    ci += 1
                        while ci < len(cms):
                            cms[ci]()
                            ci += 1
                        return psKV

                    def kv_post(i, psKV):
                        ek = kvsb.tile([128, 2, C], MDT, name="ek")
                        nc.scalar.activation(
                            out=ek[:],
                            in_=psKV[:].rearrange("p (h c) -> p h c", h=2)[:, :, 0:C],
                            func=AF.Exp,
                        )
                        eks[i] = ek
                        v2 = v2r[i % NVBUF]
                        nc.vector.tensor_copy(
                            v2[:].rearrange("p (h w) -> p h w", h=2)[:, :, 0:C],
                            psKV[:].rearrange("p (h c) -> p h c", h=2)[:, :, C : 2 * C],
                        )

                    for i in range(NSUPER):
                        psKV = kv_iter(i, i - 2)
                        kv_post(i, psKV)
                    for go in ctx_mm_list(NSUPER - 2) + ctx_mm_list(NSUPER - 1):
                        go()
                    # overlap the context epilogue with the first two Q chunks
                    # (their PSUM supertiles borrow the KV pool's slots)
                    for j in range(2):
                        psQ = pkv.tile([128, 1024], F32, name="psKV", tag="psKV")
                        q_mms_into(j, psQ)
                        eq_act(j, psQ)

                # ===== epilogue: normalize context, fold wo: W^T = ctx.T@woT =====
                rcol = [cst.tile([128, 1], F32, name=f"rcol{u}") for u in range(2)]
                ctx_sb = [cst.tile([128, C], MDT, name=f"ctx{u}") for u in range(2)]
                for u in range(2):
                    nc.vector.reciprocal(rcol[u][:], psCtx[u][:, C : C + 1])
                    nc.vector.tensor_scalar_mul(
                        out=ctx_sb[u][:], in0=psCtx[u][:, 0:C], scalar1=rcol[u][:]
                    )
                with tc.tile_pool(name="psum_w", bufs=1, space="PSUM") as pw:
                    psW = [pw.tile([128, C], F32, name=f"psW{v}") for v in range(2)]
                    for v in range(2):
                        for u in range(2):
                            nc.tensor.matmul(
                                psW[v][:],
                                ctx_sb[u][:, ts(v, 128)],
                                woT_sb[u],
                                start=(u == 0),
                                stop=(u == 1) and not use_bv,
                                skip_group_check=True,
                            )
                        if use_bv:
                            # context gains +bv[d'] per row (sum_n k = 1), so
                            # W^T += bv (X) rowsum(wo): a K=1 rank-1 matmul.
                            nc.tensor.matmul(
                                psW[v][:],
                                bv_sb[:, ts(v, 128)],
                                wosum_sb[:],
                                start=False,
                                stop=True,
                                skip_group_check=True,
                            )
                        nc.vector.tensor_copy(WT_sb[v][:], psW[v][:])

            # ============ Q phase: out = W^T.T @ softmax_head(exp(Q*s)) ============
            # Supertile layout [128, 1024]: channel-half t at cols 512t.
            # Pipelined depth 3: at iteration j the PE runs Q(j), D(j-1),
            # out(j-3) so every matmul's ACT/DVE inputs are a full iteration
            # old. D comes out of the head-block indicator matmul already
            # replicated over the 128 partitions of each half.
            with (
                tc.tile_pool(name="psq", bufs=2, space="PSUM") as pq,
                tc.tile_pool(name="psd", bufs=1, space="PSUM") as pd,
                tc.tile_pool(name="pso", bufs=1, space="PSUM") as po,
            ):
                def q_mms(j):
                    psQ = pq.tile([128, 1024], F32, name="psQ")
                    q_mms_into(j, psQ)
                    return psQ

                def d_mms(j):
                    psD = pd.tile([128, 1024], F32, name="psD")
                    for t in range(2):
                        nc.tensor.matmul(
                            psD[:, t * 512 : (t + 1) * 512],
                            hb_sb[t],
                            eqs[j][:, t * 512 : (t + 1) * 512],
                            start=True,
                            stop=True,
                        )
                    psDs[j] = psD

                def r_acts(j):
                    psD = psDs.pop(j)
                    lnD = dsb.tile([128, 1024], LDT, name="lnD")
                    nc.scalar.activation(out=lnD[:], in_=psD[:], func=AF.Ln)
                    rD = dsb.tile([128, 1024], LDT, name="rD")
                    nc.scalar.activation(out=rD[:], in_=lnD[:], func=AF.Exp, scale=-1.0)
                    rDs[j] = rD

                def q_mul(j):
                    qt = qsb.tile([128, 1024], MDT, name="qt", tag="qt")
                    nc.vector.tensor_mul(qt[:], eqs.pop(j)[:], rDs.pop(j)[:])
                    qts[j] = qt

                def out_mms(j):
                    psO = po.tile([128, 1024], F32, name="psO")
                    qt = qts.pop(j)
                    for t in range(2):
                        for u in range(2):
                            nc.tensor.matmul(
                                psO[:, t * 512 : (t + 1) * 512],
                                WT_sb[u][:, ts(t, 128)],
                                qt[:, u * 512 : (u + 1) * 512],
                                start=(u == 0),
                                stop=(u == 1),
                            )
                    psOs[j] = psO

                def store(j):
                    psO = psOs.pop(j)
                    o2 = qsb.tile([128, 1024], YDT, name="o2", tag="o2")
                    if use_bo:
                        for t in range(2):
                            nc.vector.tensor_scalar_add(
                                out=o2[:, t * 512 : (t + 1) * 512],
                                in0=psO[:, t * 512 : (t + 1) * 512],
                                scalar1=bo_sb[t][:],
                            )
                        nc.sync.dma_start(out=y4[:, j], in_=o2[:])
                    elif j == NCHUNKS - 1:
                        # final chunk: copy+store halves so the last DMA (and
                        # the tail barrier behind it) starts sooner
                        for t in range(2):
                            nc.vector.tensor_copy(
                                o2[:, t * 512 : (t + 1) * 512],
                                psO[:, t * 512 : (t + 1) * 512],
                            )
                            nc.sync.dma_start(
                                out=y4[:, j, t], in_=o2[:, t * 512 : (t + 1) * 512]
                            )
                    else:
                        nc.vector.tensor_copy(o2[:], psO[:])
                        nc.sync.dma_start(out=y4[:, j], in_=o2[:])

                for j in range(NCHUNKS + 3):
                    if 2 <= j < NCHUNKS:
                        psQ = q_mms(j)
                    if 1 <= j <= NCHUNKS:
                        d_mms(j - 1)
                    if 3 <= j <= NCHUNKS + 2:
                        out_mms(j - 3)
                    if 2 <= j < NCHUNKS:
                        eq_act(j, psQ)
                    if 1 <= j <= NCHUNKS:
                        r_acts(j - 1)
                    if 2 <= j <= NCHUNKS + 1:
                        q_mul(j - 2)
                    if 3 <= j <= NCHUNKS + 2:
                        store(j - 3)

    nc.compile()
    return nc


def _get_nc(use_bq, use_bo, use_bv, mm_dtype):
    key = (use_bq, use_bo, use_bv, str(mm_dtype))
    if key not in _CACHE:
        with _single_act_table():
            _CACHE[key] = _build(use_bq, use_bo, use_bv, mm_dtype)
    return _CACHE[key]


def kernel(x, cproj, wq, bq, wkv, bkv, wo, bo, _mm_dtype=FP16, _results_hook=None):
    x = np.ascontiguousarray(np.asarray(x, dtype=np.float32).reshape(B, C, N))
    cf = np.ascontiguousarray(np.asarray(cproj, dtype=np.float32).reshape(B, C, N))
    wq = np.asarray(wq, dtype=np.float32)
    wkv = np.asarray(wkv, dtype=np.float32)
    wo = np.asarray(wo, dtype=np.float32)
    bq = np.asarray(bq, dtype=np.float32)
    bkv = np.asarray(bkv, dtype=np.float32)
    bo = np.asarray(bo, dtype=np.float32)

    use_bq = bool(np.any(bq != 0))
    use_bo = bool(np.any(bo != 0))
    bv = bkv[C:]
    use_bv = bool(np.any(bv != 0))

    # SCALE folds into wq so the eq exp needs no ACT scale factor
    wqT = np.ascontiguousarray((SCALE * wq).T)
    wkvT = np.ascontiguousarray(wkv.T)
    woT = np.ascontiguousarray(wo.T)
    # head-block indicator: hb[c, c'] = 1 iff head(c) == head(c'), per c-half
    heads = np.arange(C) // DHEAD
    hb = (heads[:, None] == heads[None, :]).astype(np.float32)

    # packed weights per c-half u: [wkvT | wqT | woT | headblock]
    wpack = np.zeros((128, 2 * WP), np.float32)
    for u in range(2):
        r = slice(u * 128, (u + 1) * 128)
        wpack[:, u * WP : u * WP + 2 * C] = wkvT[r]
        wpack[:, u * WP + 2 * C : u * WP + 3 * C] = wqT[r]
        wpack[:, u * WP + 3 * C : u * WP + 4 * C] = woT[r]
        wpack[:, u * WP + 4 * C : u * WP + 4 * C + 128] = hb[r, r]

    nc = _get_nc(use_bq, use_bo, use_bv, _mm_dtype)

    half = _mm_dtype in (BF16, FP16)
    if half:
        mdt_np = np.float16 if _mm_dtype == FP16 else __import__("ml_dtypes").bfloat16
        x = x.astype(mdt_np)
        cf = cf.astype(mdt_np)
        wpack = wpack.astype(mdt_np)
    base = {"wpack": wpack}
    if use_bq:
        base["bq_s"] = (SCALE * bq).reshape(C, 1)
    if use_bo:
        base["bo_c"] = bo.reshape(C, 1)
    if use_bv:
        bv_r = bv.reshape(1, C)
        wosum = wo.sum(axis=1).reshape(1, C)
        if half:
            bv_r = bv_r.astype(mdt_np)
            wosum = wosum.astype(mdt_np)
        base["bv_r"] = bv_r
        base["wosum"] = wosum

    in_maps = [dict(base, x=x[b], cp=cf[b]) for b in range(B)]
    res = run_bass_kernel_spmd(nc, in_maps, list(range(NCORES)))
    if _results_hook is not None:
        _results_hook(res)
    # y[p, j, t, s] = out[t*128+p, j*512+s]
    out = np.empty((B, C, N), np.float32)
    for b in range(B):
        y3 = np.asarray(res.results[b]["y"], dtype=np.float32).reshape(
            128, NCHUNKS, 2, 512
        )
        out[b] = y3.transpose(2, 0, 1, 3).reshape(C, N)
    return out.reshape(B, C, H, W)


# revision 6
# speedup vs baseline: 1.0975x; 1.0635x over previous
"""Trainium2 Bass kernel for CrossEfficientAttention (B=8, C=256, H=W=64, 4 heads).

Sharding: data-parallel over batch B — one sample per NeuronCore, no collectives.

Per-core math (sample x_s, c_s of shape [C, N], N = H*W = 4096):
    Q  = wq @ x_s                      (+ bq, folded into the exp's ACT bias)
    KV = wkv @ c_s                     (bkv[:C] cancels exactly in softmax over N;
                                        bkv[C:] handled as a rank-1 update of W)
    k  = softmax_N(K); q = softmax_head(Q * C**-0.25)
    context = k @ V^T ; out = wo @ (context @ q) + bo

Restructured for the PE array (out = lhsT.T @ rhs, contraction over partitions):
  * KV^T computed directly in [N, C] layout by using c_s tiles as lhsT; the
    context accumulation runs two iterations behind the KV matmuls and its
    matmuls are interleaved between them so neither LDWEIGHTS nor the exp's
    ACT latency ever stalls the PE.
  * k-softmax normalizer: ones-columns appended to V^T give row sums of exp(K)
    in column 256 of the context PSUM accumulator; context rows are then scaled
    by the reciprocal column (per-partition tensor_scalar) — no transposes.
  * wo folded in early: W^T = matmul(lhsT=context, rhs=wo^T) directly in [d, o]
    layout. The per-chunk output is then just out2 = W^T.T @ q.
  * q-softmax denominators: a head-block indicator matmul (hb[c,c'] = 1 iff
    same head) sums exp(Q) directly into a partition-replicated [128, 512]
    PSUM tile per half, so 1/D = exp(-ln D) on ScalarE feeds the qt multiply
    with no separate broadcast matmul; lnD/rD ride in fp16 so the second ACT
    pass runs at the 16-bit rate.

All matmul operands are float16 (PE runs fp16 at the same 1 cycle/row as
float32r, HBM traffic and SBUF footprint halve, DVE/ACT run 2x, and the
11-bit mantissa keeps rounding noise ~4x below bf16); PSUM stays fp32, and
the output rides back to the host as fp16 (chunk-contiguous layout so store
descriptors are 2 KB/partition-row) and is upcast + reordered there. The
SCALE factor of the q softmax is pre-folded into wq on the host.
"""

import numpy as np

import concourse.bass as bass
import concourse.tile as tile
from concourse import bacc, mybir
from concourse.bass import ts
from concourse.bass_utils import run_bass_kernel_spmd

B, C, H, W = 8, 256, 64, 64
N = H * W
NHEADS = 4
DHEAD = C // NHEADS
NCORES = 8
NSUPER = N // 256          # 16 double-n-tile iterations for the KV phase
NCHUNKS = N // 512         # 8 column chunks for the Q/output phase
SCALE = float(1.0 / np.sqrt(np.sqrt(np.float32(C))))
VW = C + 2                 # V^T tile row width (256 data + 2 ones cols)
# packed weight row width per c-half: wkvT|wqT|woT|headblock
WP = 2 * C + C + C + 128
NWARM = 6                  # PE-ramp warmup matmuls (fill the initial DMA wait)

F32 = mybir.dt.float32
F32R = mybir.dt.float32r
BF16 = mybir.dt.bfloat16
FP16 = mybir.dt.float16
AF = mybir.ActivationFunctionType

_CACHE = {}


def _single_act_table():
    """Scope-patch the activation-table list so the table-load pass resolves
    both Exp and Ln to natural_log_exp_and_others (set ids stay positional,
    so only the function lists may change, not the order)."""
    import contextlib

    import concourse.bacc as cbacc
    from concourse.hw_specs import get_activation_tables

    @contextlib.contextmanager
    def scope():
        orig = cbacc.get_activation_tables

        def patched(arch):
            tabs = get_activation_tables(arch)
            return {
                k: (v if k == "natural_log_exp_and_others" else set())
                for k, v in tabs.items()
            }

        cbacc.get_activation_tables = patched
        try:
            yield
        finally:
            cbacc.get_activation_tables = orig

    return scope()


def _build(use_bq, use_bo, use_bv, mm_dtype):
    nc = bacc.Bacc("TRN2", target_bir_lowering=False, debug=False)
    MDT = mm_dtype
    HALF = MDT in (BF16, FP16)
    YDT = MDT if HALF else F32
    LDT = FP16 if HALF else F32   # lnD/rD staging dtype

    x = nc.dram_tensor("x", [C, N], MDT, kind="ExternalInput")
    cp = nc.dram_tensor("cp", [C, N], MDT, kind="ExternalInput")
    wpack = nc.dram_tensor("wpack", [128, 2 * WP], MDT, kind="ExternalInput")
    if use_bq:
        bq_s = nc.dram_tensor("bq_s", [C, 1], F32, kind="ExternalInput")
    if use_bo:
        bo_c = nc.dram_tensor("bo_c", [C, 1], F32, kind="ExternalInput")
    if use_bv:
        bv_r = nc.dram_tensor("bv_r", [1, C], MDT, kind="ExternalInput")
        wosum = nc.dram_tensor("wosum", [1, C], MDT, kind="ExternalInput")
    # chunk-contiguous output: y[p, j, t, s] = out[t*128+p, j*512+s]
    y = nc.dram_tensor("y", [128, NCHUNKS * 2 * 512], YDT, kind="ExternalOutput")
    y4 = y[:].rearrange("p (j t s) -> p j t s", j=NCHUNKS, t=2)

    with tile.TileContext(nc) as tc:
        with (
            tc.tile_pool(name="const", bufs=1) as cst,
            tc.tile_pool(name="big", bufs=1) as big,
            tc.tile_pool(name="qsb", bufs=4) as qsb,
            tc.tile_pool(name="dsb", bufs=3) as dsb,
        ):
            # HAM warmup scratch: memset on gpsimd (idle queue) so the PE can
            # start ramping as soon as the framework preamble ends.
            scratch = cst.tile([128, 512], MDT, name="scratch")
            nc.gpsimd.memset(scratch[:], 1.0)

            # --- packed weights; the KV-phase slice (wkvT) rides first ---
            wpk = cst.tile([128, 2 * WP], MDT, name="wpk")
            wpk3 = wpk[:].rearrange("p (u w) -> p u w", u=2)
            wpack3 = wpack[:].rearrange("p (u w) -> p u w", u=2)
            wkvT_sb = [wpk[:, u * WP : u * WP + 2 * C] for u in range(2)]
            wqT_sb = [wpk[:, u * WP + 2 * C : u * WP + 3 * C] for u in range(2)]
            woT_sb = [wpk[:, u * WP + 3 * C : u * WP + 4 * C] for u in range(2)]
            hb_sb = [wpk[:, u * WP + 4 * C : u * WP + 4 * C + 128] for u in range(2)]

            # weight-side loads ride the scalar DGE queue
            nc.scalar.dma_start(out=wpk3[:, :, 0 : 2 * C], in_=wpack3[:, :, 0 : 2 * C])
            if use_bq:
                bq_sb = [cst.tile([128, 1], F32, name=f"bq{u}") for u in range(2)]
                for u in range(2):
                    nc.scalar.dma_start(out=bq_sb[u][:], in_=bq_s[ts(u, 128), :])
            if use_bo:
                bo_sb = [cst.tile([128, 1], F32, name=f"bo{u}") for u in range(2)]
                for u in range(2):
                    nc.scalar.dma_start(out=bo_sb[u][:], in_=bo_c[ts(u, 128), :])
            if use_bv:
                bv_sb = cst.tile([1, C], MDT, name="bv_sb")
                nc.scalar.dma_start(out=bv_sb[:], in_=bv_r[:])
                wosum_sb = cst.tile([1, C], MDT, name="wosum_sb")
                nc.scalar.dma_start(out=wosum_sb[:], in_=wosum[:])
            nc.scalar.dma_start(out=wpk3[:, :, 2 * C : WP], in_=wpack3[:, :, 2 * C : WP])

            # cf stream leads on the sync queue (the KV phase consumes it in
            # order); x follows on the same queue so it can never compete with
            # the latency-critical early cf chunks. Chunks are >=1024 cols
            # after the first so descriptor rows stay >=2KB at 16-bit dtypes.
            cf_sb = [big.tile([128, N], MDT, name=f"cf{u}") for u in range(2)]
            xf_sb = [big.tile([128, N], MDT, name=f"xf{u}") for u in range(2)]
            for c0, c1 in ((0, 512), (512, 1024), (1024, 2048), (2048, 4096)):
                for u in range(2):
                    nc.sync.dma_start(
                        out=cf_sb[u][:, c0:c1], in_=cp[ts(u, 128), c0:c1]
                    )
            for c0, c1 in ((0, 2048), (2048, 4096)):
                for u in range(2):
                    nc.sync.dma_start(
                        out=xf_sb[u][:, c0:c1], in_=x[ts(u, 128), c0:c1]
                    )

            # persistent W^T tiles (filled in the epilogue)
            WT_sb = [cst.tile([128, C], MDT, name=f"WT{u}") for u in range(2)]

            # HAM warmup: dependency-free matmuls on scratch data keep the
            # PE busy/ramping during the initial DMA wait.
            with tc.tile_pool(name="pswarm", bufs=1, space="PSUM") as pwm:
                pswarm = pwm.tile([128, 512], F32, name="pswarm")
                for _ in range(NWARM):
                    nc.tensor.matmul(
                        pswarm[:], scratch[:, 0:128], scratch[:],
                        start=True, stop=True, skip_group_check=True,
                    )

            # manually-rotated V^T ring: ones columns pre-set once
            NVBUF = 4
            v2r = [cst.tile([128, 2 * VW], MDT, name=f"v2_{i}") for i in range(NVBUF)]
            for i in range(NVBUF):
                for h in range(2):
                    nc.vector.memset(v2r[i][:, h * VW + C : h * VW + C + 2], 1.0)

            eqs, psDs, rDs, qts, psOs = {}, {}, {}, {}, {}

            def q_mms_into(j, psQ):
                for t in range(2):
                    for u in range(2):
                        nc.tensor.matmul(
                            psQ[:, t * 512 : (t + 1) * 512],
                            wqT_sb[u][:, ts(t, 128)],
                            xf_sb[u][:, ts(j, 512)],
                            start=(u == 0),
                            stop=(u == 1),
                        )

            def eq_act(j, psQ):
                eq = qsb.tile([128, 1024], MDT, name="eq", tag="eq")
                if use_bq:
                    for t in range(2):
                        nc.scalar.activation(
                            out=eq[:, t * 512 : (t + 1) * 512],
                            in_=psQ[:, t * 512 : (t + 1) * 512],
                            func=AF.Exp,
                            bias=bq_sb[t][:],
                        )
                else:
                    nc.scalar.activation(out=eq[:], in_=psQ[:], func=AF.Exp)
                eqs[j] = eq

            # ============ KV phase: context = exp(K) @ [V^T | 1] ============
            # The context matmuls run two iterations behind the KV matmuls and
            # are interleaved between them on the PE queue.
            with tc.tile_pool(name="psum_ctx", bufs=1, space="PSUM") as pctx:
                psCtx = [
                    pctx.tile([128, C + 2], F32, name=f"psCtx{u}") for u in range(2)
                ]
                with (
                    tc.tile_pool(name="psum_kv", bufs=3, space="PSUM") as pkv,
                    tc.tile_pool(name="kvsb", bufs=3) as kvsb,
                ):
                    eks = {}

                    def ctx_mm_list(i):
                        if i < 0:
                            return []
                        ek = eks.pop(i)
                        v2 = v2r[i % NVBUF]

                        def mk(h, u):
                            def go():
                                nc.tensor.matmul(
                                    psCtx[u][:],
                                    ek[:, h, ts(u, 128)],
                                    v2[:, h * VW : (h + 1) * VW],
                                    start=(i == 0 and h == 0),
                                    stop=(i == NSUPER - 1 and h == 1),
                                    skip_group_check=True,
                                )

                            return go

                        return [mk(h, u) for h in range(2) for u in range(2)]

                    def kv_iter(i, ctx_i):
                        psKV = pkv.tile([128, 1024], F32, name="psKV")
                        cms = ctx_mm_list(ctx_i)
                        ci = 0
                        for h in range(2):
                            nt = 2 * i + h
                            for u in range(2):
                                nc.tensor.matmul(
                                    psKV[:, h * 512 : (h + 1) * 512],
                                    cf_sb[u][:, ts(nt, 128)],
                                    wkvT_sb[u],
                                    start=(u == 0),
                                    stop=(u == 1),
                                    skip_group_check=True,
                                )
                                if ci < len(cms):
                                    cms[ci]()
                                    ci += 1
                        while ci < len(cms):
                            cms[ci]()
                            ci += 1
                        return psKV

                    def kv_post(i, psKV):
                        ek = kvsb.tile([128, 2, C], MDT, name="ek")
                        nc.scalar.activation(
                            out=ek[:],
                            in_=psKV[:].rearrange("p (h c) -> p h c", h=2)[:, :, 0:C],
                            func=AF.Exp,
                        )
                        eks[i] = ek
                        v2 = v2r[i % NVBUF]
                        nc.vector.tensor_copy(
                            v2[:].rearrange("p (h w) -> p h w", h=2)[:, :, 0:C],
                            psKV[:].rearrange("p (h c) -> p h c", h=2)[:, :, C : 2 * C],
                        )

                    for i in range(NSUPER):
                        psKV = kv_iter(i, i - 2)
                        kv_post(i, psKV)
                    for go in ctx_mm_list(NSUPER - 2) + ctx_mm_list(NSUPER - 1):
                        go()
                    # overlap the context epilogue with the first two Q chunks
                    # (their PSUM supertiles borrow the KV pool's slots)
                    for j in range(2):
                        psQ = pkv.tile([128, 1024], F32, name="psKV", tag="psKV")
                        q_mms_into(j, psQ)
                        eq_act(j, psQ)

                # ===== epilogue: normalize context, fold wo: W^T = ctx.T@woT =====
                rcol = [cst.tile([128, 1], F32, name=f"rcol{u}") for u in range(2)]
                ctx_sb = [cst.tile([128, C], MDT, name=f"ctx{u}") for u in range(2)]
                for u in range(2):
                    nc.vector.reciprocal(rcol[u][:], psCtx[u][:, C : C + 1])
                    nc.vector.tensor_scalar_mul(
                        out=ctx_sb[u][:], in0=psCtx[u][:, 0:C], scalar1=rcol[u][:]
                    )
                with tc.tile_pool(name="psum_w", bufs=1, space="PSUM") as pw:
                    psW = [pw.tile([128, C], F32, name=f"psW{v}") for v in range(2)]
                    for v in range(2):
                        for u in range(2):
                            nc.tensor.matmul(
                                psW[v][:],
                                ctx_sb[u][:, ts(v, 128)],
                                woT_sb[u],
                                start=(u == 0),
                                stop=(u == 1) and not use_bv,
                                skip_group_check=True,
                            )
                        if use_bv:
                            # context gains +bv[d'] per row (sum_n k = 1), so
                            # W^T += bv (X) rowsum(wo): a K=1 rank-1 matmul.
                            nc.tensor.matmul(
                                psW[v][:],
                                bv_sb[:, ts(v, 128)],
                                wosum_sb[:],
                                start=False,
                                stop=True,
                                skip_group_check=True,
                            )
                        nc.vector.tensor_copy(WT_sb[v][:], psW[v][:])

            # ============ Q phase: out = W^T.T @ softmax_head(exp(Q*s)) ============
            # Supertile layout [128, 1024]: channel-half t at cols 512t.
            # Pipelined depth 3: at iteration j the PE runs Q(j), D(j-1),
            # out(j-3) so every matmul's ACT/DVE inputs are a full iteration
            # old. D comes out of the head-block indicator matmul already
            # replicated over the 128 partitions of each half.
            with (
                tc.tile_pool(name="psq", bufs=2, space="PSUM") as pq,
                tc.tile_pool(name="psd", bufs=1, space="PSUM") as pd,
                tc.tile_pool(name="pso", bufs=1, space="PSUM") as po,
            ):
                def q_mms(j):
                    psQ = pq.tile([128, 1024], F32, name="psQ")
                    q_mms_into(j, psQ)
                    return psQ

                def d_mms(j):
                    psD = pd.tile([128, 1024], F32, name="psD")
                    for t in range(2):
                        nc.tensor.matmul(
                            psD[:, t * 512 : (t + 1) * 512],
                            hb_sb[t],
                            eqs[j][:, t * 512 : (t + 1) * 512],
                            start=True,
                            stop=True,
                        )
                    psDs[j] = psD

                def r_acts(j):
                    psD = psDs.pop(j)
                    rD = dsb.tile([128, 1024], F32, name="rD")
                    nc.vector.reciprocal_approx_fast(out=rD[:], in_=psD[:])
                    rDs[j] = rD

                def q_mul(j):
                    qt = qsb.tile([128, 1024], MDT, name="qt", tag="qt")
                    nc.vector.tensor_mul(qt[:], eqs.pop(j)[:], rDs.pop(j)[:])
                    qts[j] = qt

                def out_mms(j):
                    psO = po.tile([128, 1024], F32, name="psO")
                    qt = qts.pop(j)
                    for t in range(2):
                        for u in range(2):
                            nc.tensor.matmul(
                                psO[:, t * 512 : (t + 1) * 512],
                                WT_sb[u][:, ts(t, 128)],
                                qt[:, u * 512 : (u + 1) * 512],
                                start=(u == 0),
                                stop=(u == 1),
                            )
                    psOs[j] = psO

                def store(j):
                    # PSUM -> SBUF narrowing copy rides the ACT engine (the
                    # 'copy' entry shares the exp/ln table) to keep DVE free
                    # for the reciprocal + qt multiply.
                    psO = psOs.pop(j)
                    o2 = qsb.tile([128, 1024], YDT, name="o2", tag="o2")
                    if use_bo:
                        for t in range(2):
                            nc.scalar.activation(
                                out=o2[:, t * 512 : (t + 1) * 512],
                                in_=psO[:, t * 512 : (t + 1) * 512],
                                func=AF.Copy,
                                bias=bo_sb[t][:],
                            )
                        nc.sync.dma_start(out=y4[:, j], in_=o2[:])
                    elif j == NCHUNKS - 1:
                        # final chunk: copy+store halves so the last DMA (and
                        # the tail barrier behind it) starts sooner
                        for t in range(2):
                            nc.scalar.activation(
                                out=o2[:, t * 512 : (t + 1) * 512],
                                in_=psO[:, t * 512 : (t + 1) * 512],
                                func=AF.Copy,
                            )
                            nc.sync.dma_start(
                                out=y4[:, j, t], in_=o2[:, t * 512 : (t + 1) * 512]
                            )
                    else:
                        nc.scalar.activation(out=o2[:], in_=psO[:], func=AF.Copy)
                        nc.sync.dma_start(out=y4[:, j], in_=o2[:])

                for j in range(NCHUNKS + 3):
                    if 2 <= j < NCHUNKS:
                        psQ = q_mms(j)
                    if 1 <= j <= NCHUNKS:
                        d_mms(j - 1)
                    if 3 <= j <= NCHUNKS + 2:
                        out_mms(j - 3)
                    if 2 <= j < NCHUNKS:
                        eq_act(j, psQ)
                    if 1 <= j <= NCHUNKS:
                        r_acts(j - 1)
                    if 2 <= j <= NCHUNKS + 1:
                        q_mul(j - 2)
                    if 3 <= j <= NCHUNKS + 2:
                        store(j - 3)

    nc.compile()
    return nc


def _get_nc(use_bq, use_bo, use_bv, mm_dtype):
    key = (use_bq, use_bo, use_bv, str(mm_dtype))
    if key not in _CACHE:
        with _single_act_table():
            _CACHE[key] = _build(use_bq, use_bo, use_bv, mm_dtype)
    return _CACHE[key]


def kernel(x, cproj, wq, bq, wkv, bkv, wo, bo, _mm_dtype=FP16, _results_hook=None):
    x = np.ascontiguousarray(np.asarray(x, dtype=np.float32).reshape(B, C, N))
    cf = np.ascontiguousarray(np.asarray(cproj, dtype=np.float32).reshape(B, C, N))
    wq = np.asarray(wq, dtype=np.float32)
    wkv = np.asarray(wkv, dtype=np.float32)
    wo = np.asarray(wo, dtype=np.float32)
    bq = np.asarray(bq, dtype=np.float32)
    bkv = np.asarray(bkv, dtype=np.float32)
    bo = np.asarray(bo, dtype=np.float32)

    use_bq = bool(np.any(bq != 0))
    use_bo = bool(np.any(bo != 0))
    bv = bkv[C:]
    use_bv = bool(np.any(bv != 0))

    # SCALE folds into wq so the eq exp needs no ACT scale factor
    wqT = np.ascontiguousarray((SCALE * wq).T)
    wkvT = np.ascontiguousarray(wkv.T)
    woT = np.ascontiguousarray(wo.T)
    # head-block indicator: hb[c, c'] = 1 iff head(c) == head(c'), per c-half
    heads = np.arange(C) // DHEAD
    hb = (heads[:, None] == heads[None, :]).astype(np.float32)

    # packed weights per c-half u: [wkvT | wqT | woT | headblock]
    wpack = np.zeros((128, 2 * WP), np.float32)
    for u in range(2):
        r = slice(u * 128, (u + 1) * 128)
        wpack[:, u * WP : u * WP + 2 * C] = wkvT[r]
        wpack[:, u * WP + 2 * C : u * WP + 3 * C] = wqT[r]
        wpack[:, u * WP + 3 * C : u * WP + 4 * C] = woT[r]
        wpack[:, u * WP + 4 * C : u * WP + 4 * C + 128] = hb[r, r]

    nc = _get_nc(use_bq, use_bo, use_bv, _mm_dtype)

    half = _mm_dtype in (BF16, FP16)
    if half:
        mdt_np = np.float16 if _mm_dtype == FP16 else __import__("ml_dtypes").bfloat16
        x = x.astype(mdt_np)
        cf = cf.astype(mdt_np)
        wpack = wpack.astype(mdt_np)
    base = {"wpack": wpack}
    if use_bq:
        base["bq_s"] = (SCALE * bq).reshape(C, 1)
    if use_bo:
        base["bo_c"] = bo.reshape(C, 1)
    if use_bv:
        bv_r = bv.reshape(1, C)
        wosum = wo.sum(axis=1).reshape(1, C)
        if half:
            bv_r = bv_r.astype(mdt_np)
            wosum = wosum.astype(mdt_np)
        base["bv_r"] = bv_r
        base["wosum"] = wosum

    in_maps = [dict(base, x=x[b], cp=cf[b]) for b in range(B)]
    res = run_bass_kernel_spmd(nc, in_maps, list(range(NCORES)))
    if _results_hook is not None:
        _results_hook(res)
    # y[p, j, t, s] = out[t*128+p, j*512+s]
    out = np.empty((B, C, N), np.float32)
    for b in range(B):
        y3 = np.asarray(res.results[b]["y"], dtype=np.float32).reshape(
            128, NCHUNKS, 2, 512
        )
        out[b] = y3.transpose(2, 0, 1, 3).reshape(C, N)
    return out.reshape(B, C, H, W)
